# revision 10
# baseline (speedup 1.0000x reference)
"""Causal attention kernel for Trainium2, 8 NeuronCores — hybrid bf16/fp8.

Problem: x[4,2048,2048] @ Wq/Wk/Wv[2048,2048] -> causal softmax attention.

Sharding: 2 cores per batch; each core owns 1024 query rows as 512-row
chunks {0,3} (even cores) / {1,2} (odd cores) for causal balance. Each core
computes Q/K/V for its OWN rows; pairwise AllGathers assemble the batch's
full K^T / V.

Precision split (the causal structure protects it): rows < 1024 (slot0 =
each core's c_lo chunk) keep the full bf16 path — their outputs average few
V rows, so quantization noise passes straight through. Rows >= 1024 (slot1
= c_hi) attend >= 1025 keys, so probability/value noise averages down by
~1/sqrt(n): that whole path (projections, scores, probs, AV) runs in
fp8 e4m3 with DoubleRow matmuls (2 contraction rows/cycle, ~1.8x measured).
Host sim (err_sim.py) puts rel err at ~1.3e-2 vs the 2e-2 gate.

fp8 scale plan: x*16, W*256 (keeps N(0,1/sqrt(D)) weights out of e4m3
subnormals), Q/K/V requantized at *8, probs exp(s)*2^-5; scales cancel
exactly in out = (p@V)/(p@ones) because the rowsum ones vector is 8.0.

Scheduling notes (Tile scheduler = priority heap over READY instructions,
per-engine strict FIFO issue, ~0.6us per DMA descriptor):
- qt8 and the slot1 score K tiles (kt8) live in OUTER pools so their SBUF
  never overlaps phase-1 pools — their loads run during phase 1 instead of
  waiting for phase-1 pool release (this was a 45us stall).
- Weight-load descriptors are interleaved into compute loops on the engine
  that paces them (scalar), 1 descriptor per tile, so no engine's FIFO
  blocks on a gated descriptor it doesn't need yet.
- x/fp8-W first so the fp8 stages start fast; stores+x on gpsimd; phase-2
  v loads on gpsimd; out stores + kt8/ktb on sync.
"""

import math

import numpy as np
import ml_dtypes

import concourse.bass as bass
import concourse.mybir as mybir
import concourse.tile as tile
from concourse import bacc
from concourse.bass import ds, ts
from concourse.bass_utils import run_bass_kernel_spmd

B, S, D = 4, 2048, 2048
P = 128
DC = D // P          # 16 contraction chunks
QROWS = 1024         # query rows per core
NCORES = 8
INV_SQRT_D = 1.0 / math.sqrt(D)

XS = 16.0            # x stored as 16*x
WS = 256.0           # W stored as 256*W
QKS = 8.0            # Q/K/V requantized as 8*val
PS_LN = -5.0 * math.log(2.0)        # probs stored as exp(s) * 2^-5
PROJ8_STORE = QKS / (XS * WS)       # psum(=XS*WS*val) -> 8*val
EXP8_SCALE = INV_SQRT_D / (QKS * QKS)

PAIRS = [[0, 1], [2, 3], [4, 5], [6, 7]]

F32 = mybir.dt.float32
BF16 = mybir.dt.bfloat16
F8 = mybir.dt.float8e4
Exp = mybir.ActivationFunctionType.Exp
Copy = mybir.ActivationFunctionType.Copy
DR = mybir.MatmulPerfMode.DoubleRow

_CACHED_NC = None


def _dr_runs(qs):
    """Slot1 AV key positions for query sub-block qs as (start, count)
    runs of DoubleRow pairs / singles."""
    runs = [(0, 2), (2, 2), (4, 2), (6, 2)]          # chunks 0,1 (full)
    n_diag = qs + 1                                   # c_hi diagonal blocks
    j = 8
    while n_diag >= 2:
        runs.append((j, 2))
        j += 2
        n_diag -= 2
    if n_diag:
        runs.append((j, 1))
    runs += [(12, 2), (14, 2)]                        # chunk 2 (full)
    return runs


def build_nc():
    global _CACHED_NC
    if _CACHED_NC is not None:
        return _CACHED_NC
    nc = bacc.Bacc(trn_type="TRN2", target_bir_lowering=False, debug=False,
                   num_devices=NCORES)

    xtb_d = nc.dram_tensor("xtb", [D, 512], BF16, kind="ExternalInput")
    xt8_d = nc.dram_tensor("xt8", [D, 512], F8, kind="ExternalInput")
    wqb_d = nc.dram_tensor("wqb", [DC, P, DC, P], BF16, kind="ExternalInput")
    wkb_d = nc.dram_tensor("wkb", [DC, P, DC, P], BF16, kind="ExternalInput")
    wq8_d = nc.dram_tensor("wq8", [DC, P, DC, P], F8, kind="ExternalInput")
    wk8_d = nc.dram_tensor("wk8", [DC, P, DC, P], F8, kind="ExternalInput")
    wvb_d = nc.dram_tensor("wvb", [4, 2, P, 8, 512], BF16, kind="ExternalInput")
    wv8_d = nc.dram_tensor("wv8", [DC, P, 4, 512], F8, kind="ExternalInput")
    mk0_d = nc.dram_tensor("mk0", [P, 8, 512], BF16, kind="ExternalInput")
    mk1_d = nc.dram_tensor("mk1", [P, 8, 512], F8, kind="ExternalInput")
    out_d = nc.dram_tensor("out", [QROWS, D], F32, kind="ExternalOutput")

    with tile.TileContext(nc) as tc:
        with (
            tc.tile_pool(name="dram", bufs=1, space="DRAM") as dpool,
            tc.tile_pool(name="ps_a", bufs=4, space="PSUM") as ps_a,
            tc.tile_pool(name="ps_b", bufs=4, space="PSUM") as ps_b,
            tc.tile_pool(name="qt8p", bufs=1) as qt8_pool,
        ):
            qTb = dpool.tile([P, DC, 512], BF16, tag="qTb")
            kTb_own = dpool.tile([P, DC, 512], BF16, tag="kTbo")
            kT8_lo = dpool.tile([P, DC, 512], F8, tag="kT8lo")
            kT8_hi = dpool.tile([P, DC, 512], F8, tag="kT8hi")
            kgb = dpool.tile([2, P, DC, 512], BF16, tag="kgb")
            kg8A = dpool.tile([2, P, DC, 512], F8, tag="kg8A")
            kg8B = dpool.tile([2, P, DC, 512], F8, tag="kg8B")
            vvb_own = dpool.tile([4, P, D], BF16, tag="vvbo")
            vv8_lo = dpool.tile([4, P, D], F8, tag="vv8lo")
            vv8_hi = dpool.tile([4, P, D], F8, tag="vv8hi")
            vgb = dpool.tile([2, 4, P, D], BF16, tag="vgb")
            vg8A = dpool.tile([2, 4, P, D], F8, tag="vg8A")
            vg8B = dpool.tile([2, 4, P, D], F8, tag="vg8B")

            qt8 = qt8_pool.tile([P, DC, 512], F8, tag="qt8")

            def gather(src, dst):
                nc.gpsimd.collective_compute(
                    "AllGather", mybir.AluOpType.bypass,
                    replica_groups=PAIRS, ins=[src.opt()], outs=[dst.opt()],
                )

            # ---------------- phase 1: projections -----------------------
            with (
                tc.tile_pool(name="xt", bufs=1) as xt_pool,
                tc.tile_pool(name="w8", bufs=8) as w8_pool,
                tc.tile_pool(name="wb", bufs=12) as wb_pool,
                tc.tile_pool(name="st", bufs=8) as st_pool,
                tc.tile_pool(name="st8", bufs=8) as st8_pool,
            ):
                # x chunks on gpsimd (idle early); fp8 first
                xt8 = xt_pool.tile([P, DC, 512], F8, tag="xt8", name="xt8")
                for dc in range(DC):
                    nc.gpsimd.dma_start(xt8[:, dc, :],
                                        xt8_d.ap()[ds(dc * P, P)])
                # wk8[0] split 4-way for a fast first matmul; rest 1 desc
                wk8_pre = [w8_pool.tile([P, DC, P], F8, tag="w8", name="wk80")]
                for j in range(4):
                    nc.sync.dma_start(wk8_pre[0][:, ts(j, 4), :],
                                      wk8_d.ap()[0][:, ts(j, 4), :])
                for m in range(1, DC):
                    wt = w8_pool.tile([P, DC, P], F8, tag="w8", name=f"wk8{m}")
                    nc.sync.dma_start(wt[:], wk8_d.ap()[m])
                    wk8_pre.append(wt)
                xtb = xt_pool.tile([P, DC, 512], BF16, tag="xtb", name="xtb")
                for dc in range(DC):
                    nc.gpsimd.dma_start(xtb[:, dc, :],
                                        xtb_d.ap()[ds(dc * P, P)])
                wkb_pre = [wb_pool.tile([P, DC, P], BF16, tag="w",
                                        name=f"wkb{m}") for m in range(DC)]

                # --- K c_hi (fp8 DR); wkb issue paced by the scalar copies
                for m in range(DC):
                    nc.scalar.dma_start(wkb_pre[m][:], wkb_d.ap()[m])
                    ps = ps_b.tile([P, 512], F32, tag="ps")
                    for dcp in range(DC // 2):
                        nc.tensor.matmul(
                            ps[:], lhsT=wk8_pre[m][:, 2 * dcp:2 * dcp + 2, :],
                            rhs=xt8[:, 2 * dcp:2 * dcp + 2, :],
                            start=(dcp == 0), stop=(dcp == DC // 2 - 1),
                            perf_mode=DR,
                        )
                    s8 = st8_pool.tile([P, 512], F8, tag="s8")
                    nc.scalar.activation(s8[:], ps[:], Copy, scale=PROJ8_STORE)
                    nc.gpsimd.dma_start(kT8_hi[:, m, :], s8[:])
                gather(kT8_hi, kg8B)

                # --- K c_lo (bf16), dual store bf16 + fp8
                for m in range(DC):
                    ps = ps_a.tile([P, 512], F32, tag="ps")
                    for dc in range(DC):
                        nc.tensor.matmul(
                            ps[:], lhsT=wkb_pre[m][:, dc, :], rhs=xtb[:, dc, :],
                            start=(dc == 0), stop=(dc == DC - 1),
                        )
                    st = st_pool.tile([P, 512], BF16, tag="st")
                    nc.vector.tensor_copy(st[:], ps[:])
                    s8 = st8_pool.tile([P, 512], F8, tag="s8")
                    nc.vector.tensor_scalar_mul(s8[:], ps[:], QKS)
                    nc.gpsimd.dma_start(kTb_own[:, m, :], st[:])
                    nc.gpsimd.dma_start(kT8_lo[:, m, :], s8[:])
                gather(kTb_own, kgb)
                gather(kT8_lo, kg8A)

                # --- V c_hi (fp8 DR) then V c_lo; wqb/wvb paced on scalar
                with (
                    tc.tile_pool(name="wv8", bufs=1) as wv8_pool,
                    tc.tile_pool(name="wv", bufs=4) as wv_pool,
                    tc.tile_pool(name="sv", bufs=6) as sv_pool,
                    tc.tile_pool(name="sv8", bufs=6) as sv8_pool,
                ):
                    wv8 = wv8_pool.tile([P, DC, 4, 512], F8, tag="wv8",
                                        name="wv8")
                    for dc in range(DC):
                        nc.sync.dma_start(wv8[:, dc, :, :], wv8_d.ap()[dc])
                    wqb_pre = [wb_pool.tile([P, DC, P], BF16, tag="w",
                                            name=f"wqb{m}") for m in range(DC)]
                    wv_tiles = [wv_pool.tile([P, 8, 512], BF16, tag="wv",
                                             name=f"wvb{n}{hb}")
                                for n in range(4) for hb in range(2)]

                    it = 0
                    for g in range(4):
                        for n in range(4):
                            # paced issue: 1 wqb + 1 wvb descriptor per iter
                            if it < DC:
                                nc.scalar.dma_start(wqb_pre[it][:],
                                                    wqb_d.ap()[it])
                            if it < 8:
                                nc.scalar.dma_start(
                                    wv_tiles[it][:],
                                    wvb_d.ap()[it // 2, it % 2])
                            it += 1
                            ps = ps_b.tile([P, 512], F32, tag="ps")
                            for dcp in range(DC // 2):
                                nc.tensor.matmul(
                                    ps[:],
                                    lhsT=xt8[:, 2 * dcp:2 * dcp + 2, ts(g, P)],
                                    rhs=wv8[:, 2 * dcp:2 * dcp + 2, n, :],
                                    start=(dcp == 0), stop=(dcp == DC // 2 - 1),
                                    perf_mode=DR,
                                )
                            sv8 = sv8_pool.tile([P, 512], F8, tag="sv8")
                            nc.vector.tensor_scalar_mul(sv8[:], ps[:],
                                                        PROJ8_STORE)
                            nc.gpsimd.dma_start(vv8_hi[g, :, ts(n, 512)],
                                                sv8[:])
                    gather(vv8_hi, vg8B)

                    for n in range(4):
                        wva, wvb_t = wv_tiles[2 * n], wv_tiles[2 * n + 1]
                        for g in range(4):
                            ps = ps_a.tile([P, 512], F32, tag="ps")
                            for dc in range(DC):
                                w = wva if dc < 8 else wvb_t
                                nc.tensor.matmul(
                                    ps[:], lhsT=xtb[:, dc, ts(g, P)],
                                    rhs=w[:, dc % 8, :],
                                    start=(dc == 0), stop=(dc == DC - 1),
                                )
                            sv = sv_pool.tile([P, 512], BF16, tag="sv")
                            nc.vector.tensor_copy(sv[:], ps[:])
                            sv8 = sv8_pool.tile([P, 512], F8, tag="sv8")
                            nc.vector.tensor_scalar_mul(sv8[:], ps[:], QKS)
                            nc.gpsimd.dma_start(vvb_own[g, :, ts(n, 512)],
                                                sv[:])
                            nc.gpsimd.dma_start(vv8_lo[g, :, ts(n, 512)],
                                                sv8[:])
                    gather(vvb_own, vgb)
                    gather(vv8_lo, vg8A)

                # --- Q c_hi (fp8 DR) straight into SBUF qt8
                wq8_pre = []
                for m in range(DC):
                    wt = w8_pool.tile([P, DC, P], F8, tag="w8", name=f"wq8{m}")
                    nc.sync.dma_start(wt[:], wq8_d.ap()[m])
                    wq8_pre.append(wt)
                for m in range(DC):
                    ps = ps_b.tile([P, 512], F32, tag="ps")
                    for dcp in range(DC // 2):
                        nc.tensor.matmul(
                            ps[:], lhsT=wq8_pre[m][:, 2 * dcp:2 * dcp + 2, :],
                            rhs=xt8[:, 2 * dcp:2 * dcp + 2, :],
                            start=(dcp == 0), stop=(dcp == DC // 2 - 1),
                            perf_mode=DR,
                        )
                    nc.scalar.activation(qt8[:, m, :], ps[:], Copy,
                                         scale=PROJ8_STORE)
                # --- Q c_lo (bf16) via DRAM (reload hides under slot1 scores)
                for m in range(DC):
                    ps = ps_a.tile([P, 512], F32, tag="ps")
                    for dc in range(DC):
                        nc.tensor.matmul(
                            ps[:], lhsT=wqb_pre[m][:, dc, :], rhs=xtb[:, dc, :],
                            start=(dc == 0), stop=(dc == DC - 1),
                        )
                    st = st_pool.tile([P, 512], BF16, tag="st")
                    nc.scalar.copy(st[:], ps[:])
                    nc.gpsimd.dma_start(qTb[:, m, :], st[:])

            # ---------------- phase 2: attention ----------------
            with (
                tc.tile_pool(name="pt0", bufs=1) as pt0_pool,
                tc.tile_pool(name="pt1", bufs=1) as pt1_pool,
                tc.tile_pool(name="mk", bufs=1) as mk_pool,
                tc.tile_pool(name="vb", bufs=1) as vb_pool,
                tc.tile_pool(name="ktb", bufs=6) as ktb_pool,
                tc.tile_pool(name="kt8", bufs=16) as kt8_pool,
                tc.tile_pool(name="qtb", bufs=1) as qtb_pool,
                tc.tile_pool(name="one", bufs=1) as one_pool,
                tc.tile_pool(name="sc", bufs=4) as sc_pool,
                tc.tile_pool(name="ob", bufs=4) as ob_pool,
            ):
                qtb = qtb_pool.tile([P, DC, 512], BF16, tag="qtb", name="qtb")
                for j in range(4):
                    nc.scalar.dma_start(qtb[:, ts(j, 4), :],
                                        qTb[:, ts(j, 4), :])
                mk0 = mk_pool.tile([P, 8, 512], BF16, tag="mk0")
                nc.scalar.dma_start(mk0[:, :4, :], mk0_d.ap()[:, :4, :])
                nc.scalar.dma_start(mk0[:, 4:, :], mk0_d.ap()[:, 4:, :])
                mk1 = mk_pool.tile([P, 8, 512], F8, tag="mk1")
                nc.scalar.dma_start(mk1[:], mk1_d.ap()[:, :, :])
                ones_b = one_pool.tile([P, 1], BF16, tag="onb")
                nc.vector.memset(ones_b[:], 1.0)
                ones8 = one_pool.tile([P, 1], F8, tag="on8")
                nc.vector.memset(ones8[:], QKS)
                bln = one_pool.tile([P, 1], F32, tag="bln")
                nc.vector.memset(bln[:], PS_LN)

                pt1 = pt1_pool.tile([P, DC, 512], F8, tag="pt1")
                pt0 = pt0_pool.tile([P, 8, 512], BF16, tag="pt0")

                # v loads on gpsimd (idle in phase 2); 2 desc per position
                vbig = vb_pool.tile([P, 8, D], BF16, tag="vb", name="vb")
                for j in range(8):
                    src = vgb[j // 4, j % 4]
                    nc.gpsimd.dma_start(vbig[:, j, :1024], src[:, :1024])
                    nc.gpsimd.dma_start(vbig[:, j, 1024:], src[:, 1024:])
                v8 = vb_pool.tile([P, DC, D], F8, tag="v8", name="v8")
                for pos in range(DC):
                    src = (vg8A if pos < 8 else vg8B)[(pos % 8) // 4, pos % 4]
                    nc.gpsimd.dma_start(v8[:, pos, :1024], src[:, :1024])
                    nc.gpsimd.dma_start(v8[:, pos, 1024:], src[:, 1024:])

                # --- slot1 scores (fp8 DR), all 16 key positions
                for pos in range(DC):
                    kt8_t = kt8_pool.tile([P, DC, P], F8, tag="kt8",
                                          name=f"kt8{pos}")
                    ksrc = kg8A if pos < 8 else kg8B
                    o = pos % 8
                    nc.sync.dma_start(kt8_t[:, :8, :],
                                      ksrc[o // 4][:, :8, ts(o % 4, P)])
                    nc.sync.dma_start(kt8_t[:, 8:, :],
                                      ksrc[o // 4][:, 8:, ts(o % 4, P)])
                    ps = ps_b.tile([P, 512], F32, tag="ps")
                    for dcp in range(DC // 2):
                        nc.tensor.matmul(
                            ps[:], lhsT=kt8_t[:, 2 * dcp:2 * dcp + 2, :],
                            rhs=qt8[:, 2 * dcp:2 * dcp + 2, :],
                            start=(dcp == 0), stop=(dcp == DC // 2 - 1),
                            perf_mode=DR,
                        )
                    nc.scalar.activation(pt1[:, pos, :], ps[:], Exp,
                                         scale=EXP8_SCALE, bias=bln[:])

                # --- slot0 scores (bf16), key positions 0-7
                for pos in range(8):
                    ktb_t = ktb_pool.tile([P, DC, P], BF16, tag="ktb",
                                          name=f"ktb{pos}")
                    nc.sync.dma_start(ktb_t[:, :8, :],
                                      kgb[pos // 4][:, :8, ts(pos % 4, P)])
                    nc.sync.dma_start(ktb_t[:, 8:, :],
                                      kgb[pos // 4][:, 8:, ts(pos % 4, P)])
                    ps = ps_a.tile([P, 512], F32, tag="ps")
                    for dc in range(DC):
                        nc.tensor.matmul(
                            ps[:], lhsT=ktb_t[:, dc, :], rhs=qtb[:, dc, :],
                            start=(dc == 0), stop=(dc == DC - 1),
                        )
                    nc.scalar.activation(pt0[:, pos, :], ps[:], Exp,
                                         scale=INV_SQRT_D)

                # --- masks: slot0 all 8; slot1 positions 8-15 only
                for j in range(8):
                    nc.vector.tensor_mul(pt0[:, j, :], pt0[:, j, :],
                                         mk0[:, j, :])
                for j in range(8):
                    nc.vector.tensor_mul(pt1[:, 8 + j, :], pt1[:, 8 + j, :],
                                         mk1[:, j, :])

                # --- slot0 AV (bf16): out rows 0-511
                for qs in range(4):
                    idx = list(range(4)) + list(range(4, 5 + qs))
                    plt = ps_a.tile([P, 512], F32, tag="ps", name="pl0")
                    pl = plt[:, :1]
                    for i, j in enumerate(idx):
                        nc.tensor.matmul(
                            pl[:], lhsT=pt0[:, j, ts(qs, P)], rhs=ones_b[:],
                            start=(i == 0), stop=(i == len(idx) - 1),
                        )
                    rl = sc_pool.tile([P, 1], F32, tag="rl")
                    nc.vector.reciprocal(rl[:], pl[:])
                    for n in range(4):
                        pav = ps_a.tile([P, 512], F32, tag="ps", name="pav0")
                        for i, j in enumerate(idx):
                            nc.tensor.matmul(
                                pav[:], lhsT=pt0[:, j, ts(qs, P)],
                                rhs=vbig[:, j, ts(n, 512)],
                                start=(i == 0), stop=(i == len(idx) - 1),
                            )
                        ob = ob_pool.tile([P, 512], F32, tag="ob")
                        if n % 2 == 0:
                            nc.vector.tensor_scalar_mul(ob[:], pav[:], rl[:])
                        else:
                            nc.scalar.activation(ob[:], pav[:], Copy,
                                                 scale=rl[:])
                        nc.sync.dma_start(
                            out_d.ap()[ds(qs * P, P), ts(n, 512)], ob[:])

                # --- slot1 AV (fp8 DR pairs): out rows 512-1023
                for qs in range(4):
                    idx = (list(range(8)) + list(range(8, 9 + qs))
                           + list(range(12, 16)))
                    runs = _dr_runs(qs)
                    plt = ps_b.tile([P, 512], F32, tag="ps", name="pl1")
                    pl = plt[:, :1]
                    for i, j in enumerate(idx):
                        nc.tensor.matmul(
                            pl[:], lhsT=pt1[:, j, ts(qs, P)], rhs=ones8[:],
                            start=(i == 0), stop=(i == len(idx) - 1),
                        )
                    rl = sc_pool.tile([P, 1], F32, tag="rl")
                    nc.vector.reciprocal(rl[:], pl[:])
                    for n in range(4):
                        pav = ps_b.tile([P, 512], F32, tag="ps", name="pav1")
                        for i, (j, cnt) in enumerate(runs):
                            if cnt == 2:
                                nc.tensor.matmul(
                                    pav[:], lhsT=pt1[:, j:j + 2, ts(qs, P)],
                                    rhs=v8[:, j:j + 2, ts(n, 512)],
                                    start=(i == 0), stop=(i == len(runs) - 1),
                                    perf_mode=DR,
                                )
                            else:
                                nc.tensor.matmul(
                                    pav[:], lhsT=pt1[:, j, ts(qs, P)],
                                    rhs=v8[:, j, ts(n, 512)],
                                    start=(i == 0), stop=(i == len(runs) - 1),
                                )
                        ob = ob_pool.tile([P, 512], F32, tag="ob")
                        if n % 2 == 0:
                            nc.vector.tensor_scalar_mul(ob[:], pav[:], rl[:])
                        else:
                            nc.scalar.activation(ob[:], pav[:], Copy,
                                                 scale=rl[:])
                        nc.sync.dma_start(
                            out_d.ap()[ds(512 + qs * P, P), ts(n, 512)], ob[:])

    nc.compile()
    _CACHED_NC = nc
    return nc


def _host_prep(x, Wq, Wk, Wv):
    """Build per-core input maps (host-side layout + quantization)."""
    E4 = ml_dtypes.float8_e4m3
    BFnp = ml_dtypes.bfloat16

    def wb_layout(W):
        return np.ascontiguousarray(
            W.reshape(DC, P, DC, P).transpose(2, 1, 0, 3)).astype(BFnp)

    def w8_layout(W):
        return np.ascontiguousarray(
            (W * WS).reshape(DC, P, DC, P).transpose(2, 1, 0, 3)).astype(E4)

    wqb_h, wkb_h = wb_layout(Wq), wb_layout(Wk)
    wq8_h, wk8_h = w8_layout(Wq), w8_layout(Wk)
    wvb_h = np.ascontiguousarray(
        Wv.reshape(2, 8, P, 4, 512).transpose(3, 0, 2, 1, 4)).astype(BFnp)
    wv8_h = np.ascontiguousarray(
        (Wv * WS).reshape(DC, P, 4, 512)).astype(E4)

    k_in = np.arange(P, dtype=np.int64)[:, None]        # [P, 1]
    q_in = np.arange(512, dtype=np.int64)[None, :]      # [1, 512]

    def build_masks(h):
        m0 = np.zeros((P, 8, 512), dtype=BFnp)
        for j in range(8):
            m0[:, j, :] = (j * P + k_in) <= (h * 512 + q_in)
        m1 = np.zeros((P, 8, 512), dtype=E4)
        q_off = 1536 if h == 0 else 1024
        for j in range(8):
            tkb = 12 + j if j < 4 else 4 + j
            m1[:, j, :] = (tkb * P + k_in) <= (q_off + q_in)
        return m0, m1

    masks = [build_masks(0), build_masks(1)]

    in_maps = []
    for core in range(NCORES):
        b, h = divmod(core, 2)
        c_lo, c_hi = h, 3 - h
        xt = x[b].T                                      # [D, S] view
        xlo = xt[:, c_lo * 512:(c_lo + 1) * 512]
        xhi = xt[:, c_hi * 512:(c_hi + 1) * 512]
        m0, m1 = masks[h]
        in_maps.append({
            "xtb": np.ascontiguousarray(xlo).astype(BFnp),
            "xt8": np.ascontiguousarray(xhi * XS).astype(E4),
            "wqb": wqb_h, "wkb": wkb_h, "wq8": wq8_h, "wk8": wk8_h,
            "wvb": wvb_h, "wv8": wv8_h, "mk0": m0, "mk1": m1,
        })
    return in_maps


def run(x, Wq, Wk, Wv, trace=False):
    x = np.asarray(x, dtype=np.float32)
    Wq = np.asarray(Wq, dtype=np.float32)
    Wk = np.asarray(Wk, dtype=np.float32)
    Wv = np.asarray(Wv, dtype=np.float32)
    nc = build_nc()
    in_maps = _host_prep(x, Wq, Wk, Wv)
    res = run_bass_kernel_spmd(nc, in_maps, core_ids=list(range(NCORES)),
                               trace=trace)
    out = np.empty((B, S, D), dtype=np.float32)
    for core in range(NCORES):
        b, h = divmod(core, 2)
        c_lo, c_hi = h, 3 - h
        o = res.results[core]["out"]
        out[b, c_lo * 512:(c_lo + 1) * 512] = o[:512]
        out[b, c_hi * 512:(c_hi + 1) * 512] = o[512:]
    return out, res


def kernel(x, Wq, Wk, Wv):
    out, _ = run(x, Wq, Wk, Wv)
    return out


if __name__ == "__main__":
    build_nc()
    print("build + compile OK")


# revision 11
# speedup vs baseline: 1.0507x; 1.0507x over previous
"""Causal attention kernel for Trainium2, 8 NeuronCores — hybrid bf16/fp8.

Problem: x[4,2048,2048] @ Wq/Wk/Wv[2048,2048] -> causal softmax attention.

Sharding: 2 cores per batch; each core owns 1024 query rows as 512-row
chunks {0,3} (even cores) / {1,2} (odd cores) for causal balance. Each core
computes Q/K/V for its OWN rows; pairwise AllGathers assemble the batch's
full K^T / V.

Precision split (the causal structure protects it): rows < 1024 (slot0 =
each core's c_lo chunk) keep the full bf16 path — their outputs average few
V rows, so quantization noise passes straight through. Rows >= 1024 (slot1
= c_hi) attend >= 1025 keys, so probability/value noise averages down by
~1/sqrt(n): that whole path (projections, scores, probs, AV) runs in
fp8 e4m3 with DoubleRow matmuls (2 contraction rows/cycle, ~1.8x measured).
Host sim (err_sim.py) puts rel err at ~1.3e-2 vs the 2e-2 gate.

fp8 scale plan: x*16, W*256 (keeps N(0,1/sqrt(D)) weights out of e4m3
subnormals), Q/K/V requantized at *8, probs exp(s)*2^-5; scales cancel
exactly in out = (p@V)/(p@ones) because the rowsum ones vector is 8.0.

Scheduling notes (Tile scheduler = priority heap over READY instructions,
per-engine strict FIFO issue, ~0.6us per DMA descriptor):
- qt8 and the slot1 score K tiles (kt8) live in OUTER pools so their SBUF
  never overlaps phase-1 pools — their loads run during phase 1 instead of
  waiting for phase-1 pool release (this was a 45us stall).
- Weight-load descriptors are interleaved into compute loops on the engine
  that paces them (scalar), 1 descriptor per tile, so no engine's FIFO
  blocks on a gated descriptor it doesn't need yet.
- x/fp8-W first so the fp8 stages start fast; stores+x on gpsimd; phase-2
  v loads on gpsimd; out stores + kt8/ktb on sync.
"""

import math

import numpy as np
import ml_dtypes

import concourse.bass as bass
import concourse.mybir as mybir
import concourse.tile as tile
from concourse import bacc
from concourse.bass import ds, ts
from concourse.bass_utils import run_bass_kernel_spmd

B, S, D = 4, 2048, 2048
P = 128
DC = D // P          # 16 contraction chunks
QROWS = 1024         # query rows per core
NCORES = 8
INV_SQRT_D = 1.0 / math.sqrt(D)

XS = 16.0            # x stored as 16*x
WS = 256.0           # W stored as 256*W
QKS = 8.0            # Q/K/V requantized as 8*val
PS_LN = -5.0 * math.log(2.0)        # probs stored as exp(s) * 2^-5
PROJ8_STORE = QKS / (XS * WS)       # psum(=XS*WS*val) -> 8*val
EXP8_SCALE = INV_SQRT_D / (QKS * QKS)

PAIRS = [[0, 1], [2, 3], [4, 5], [6, 7]]

F32 = mybir.dt.float32
BF16 = mybir.dt.bfloat16
F8 = mybir.dt.float8e4
Exp = mybir.ActivationFunctionType.Exp
Copy = mybir.ActivationFunctionType.Copy
DR = mybir.MatmulPerfMode.DoubleRow

_CACHED_NC = None


def _dr_runs(qs):
    """Slot1 AV key positions for query sub-block qs as (start, count)
    runs of DoubleRow pairs / singles."""
    runs = [(0, 2), (2, 2), (4, 2), (6, 2)]          # chunks 0,1 (full)
    n_diag = qs + 1                                   # c_hi diagonal blocks
    j = 8
    while n_diag >= 2:
        runs.append((j, 2))
        j += 2
        n_diag -= 2
    if n_diag:
        runs.append((j, 1))
    runs += [(12, 2), (14, 2)]                        # chunk 2 (full)
    return runs


def build_nc():
    global _CACHED_NC
    if _CACHED_NC is not None:
        return _CACHED_NC
    nc = bacc.Bacc(trn_type="TRN2", target_bir_lowering=False, debug=False,
                   num_devices=NCORES)

    xtb_d = nc.dram_tensor("xtb", [D, 512], BF16, kind="ExternalInput")
    xt8_d = nc.dram_tensor("xt8", [D, 512], F8, kind="ExternalInput")
    wqb_d = nc.dram_tensor("wqb", [DC, P, DC, P], BF16, kind="ExternalInput")
    wkb_d = nc.dram_tensor("wkb", [DC, P, DC, P], BF16, kind="ExternalInput")
    wq8_d = nc.dram_tensor("wq8", [DC, P, DC, P], F8, kind="ExternalInput")
    wk8_d = nc.dram_tensor("wk8", [DC, P, DC, P], F8, kind="ExternalInput")
    wvb_d = nc.dram_tensor("wvb", [4, 2, P, 8, 512], BF16, kind="ExternalInput")
    wv8_d = nc.dram_tensor("wv8", [DC, P, 4, 512], F8, kind="ExternalInput")
    mk0_d = nc.dram_tensor("mk0", [P, 8, 512], BF16, kind="ExternalInput")
    mk1_d = nc.dram_tensor("mk1", [P, 8, 512], F8, kind="ExternalInput")
    out_d = nc.dram_tensor("out", [QROWS, D], F32, kind="ExternalOutput")

    with tile.TileContext(nc) as tc:
        with (
            tc.tile_pool(name="dram", bufs=1, space="DRAM") as dpool,
            tc.tile_pool(name="ps", bufs=8, space="PSUM") as ps_all,
            tc.tile_pool(name="qt8p", bufs=1) as qt8_pool,
        ):
            qTb = dpool.tile([P, DC, 512], BF16, tag="qTb")
            kTb_own = dpool.tile([P, DC, 512], BF16, tag="kTbo")
            kT8_lo = dpool.tile([P, DC, 512], F8, tag="kT8lo")
            kT8_hi = dpool.tile([P, DC, 512], F8, tag="kT8hi")
            kgb = dpool.tile([2, P, DC, 512], BF16, tag="kgb")
            kg8A = dpool.tile([2, P, DC, 512], F8, tag="kg8A")
            kg8B = dpool.tile([2, P, DC, 512], F8, tag="kg8B")
            vvb_own = dpool.tile([4, P, D], BF16, tag="vvbo")
            vv8_lo = dpool.tile([4, P, D], F8, tag="vv8lo")
            vv8_hi = dpool.tile([4, P, D], F8, tag="vv8hi")
            vgb = dpool.tile([2, 4, P, D], BF16, tag="vgb")
            vg8A = dpool.tile([2, 4, P, D], F8, tag="vg8A")
            vg8B = dpool.tile([2, 4, P, D], F8, tag="vg8B")

            qt8 = qt8_pool.tile([P, DC, 512], F8, tag="qt8")

            def gather(src, dst):
                nc.gpsimd.collective_compute(
                    "AllGather", mybir.AluOpType.bypass,
                    replica_groups=PAIRS, ins=[src.opt()], outs=[dst.opt()],
                )

            # ---------------- phase 1: projections -----------------------
            with (
                tc.tile_pool(name="xt", bufs=1) as xt_pool,
                tc.tile_pool(name="w8", bufs=8) as w8_pool,
                tc.tile_pool(name="wb", bufs=12) as wb_pool,
                tc.tile_pool(name="st", bufs=8) as st_pool,
                tc.tile_pool(name="st8", bufs=8) as st8_pool,
            ):
                # x chunks on gpsimd (idle early); fp8 first
                xt8 = xt_pool.tile([P, DC, 512], F8, tag="xt8", name="xt8")
                for dc in range(DC):
                    nc.gpsimd.dma_start(xt8[:, dc, :],
                                        xt8_d.ap()[ds(dc * P, P)])
                # wk8[0] split 4-way for a fast first matmul; rest 1 desc
                wk8_pre = [w8_pool.tile([P, DC, P], F8, tag="w8", name="wk80")]
                for j in range(4):
                    nc.sync.dma_start(wk8_pre[0][:, ts(j, 4), :],
                                      wk8_d.ap()[0][:, ts(j, 4), :])
                for m in range(1, DC):
                    wt = w8_pool.tile([P, DC, P], F8, tag="w8", name=f"wk8{m}")
                    nc.sync.dma_start(wt[:], wk8_d.ap()[m])
                    wk8_pre.append(wt)
                xtb = xt_pool.tile([P, DC, 512], BF16, tag="xtb", name="xtb")
                for dc in range(DC):
                    nc.gpsimd.dma_start(xtb[:, dc, :],
                                        xtb_d.ap()[ds(dc * P, P)])
                wkb_pre = [wb_pool.tile([P, DC, P], BF16, tag="w",
                                        name=f"wkb{m}") for m in range(DC)]

                # --- K c_hi (fp8 DR); wkb issue paced by the scalar copies
                for m in range(DC):
                    nc.scalar.dma_start(wkb_pre[m][:], wkb_d.ap()[m])
                    ps = ps_all.tile([P, 512], F32, tag="ps")
                    for dcp in range(DC // 2):
                        nc.tensor.matmul(
                            ps[:], lhsT=wk8_pre[m][:, 2 * dcp:2 * dcp + 2, :],
                            rhs=xt8[:, 2 * dcp:2 * dcp + 2, :],
                            start=(dcp == 0), stop=(dcp == DC // 2 - 1),
                            perf_mode=DR,
                        )
                    s8 = st8_pool.tile([P, 512], F8, tag="s8")
                    nc.scalar.activation(s8[:], ps[:], Copy, scale=PROJ8_STORE)
                    nc.gpsimd.dma_start(kT8_hi[:, m, :], s8[:])
                gather(kT8_hi, kg8B)

                # --- K c_lo (bf16), dual store bf16 + fp8
                for m in range(DC):
                    ps = ps_all.tile([P, 512], F32, tag="ps")
                    for dc in range(DC):
                        nc.tensor.matmul(
                            ps[:], lhsT=wkb_pre[m][:, dc, :], rhs=xtb[:, dc, :],
                            start=(dc == 0), stop=(dc == DC - 1),
                        )
                    st = st_pool.tile([P, 512], BF16, tag="st")
                    nc.vector.tensor_copy(st[:], ps[:])
                    s8 = st8_pool.tile([P, 512], F8, tag="s8")
                    nc.vector.tensor_scalar_mul(s8[:], ps[:], QKS)
                    nc.gpsimd.dma_start(kTb_own[:, m, :], st[:])
                    nc.gpsimd.dma_start(kT8_lo[:, m, :], s8[:])
                gather(kTb_own, kgb)
                gather(kT8_lo, kg8A)

                # --- V c_hi (fp8 DR) then V c_lo; wqb/wvb paced on scalar
                with (
                    tc.tile_pool(name="wv8", bufs=1) as wv8_pool,
                    tc.tile_pool(name="wv", bufs=4) as wv_pool,
                    tc.tile_pool(name="sv", bufs=6) as sv_pool,
                    tc.tile_pool(name="sv8", bufs=6) as sv8_pool,
                ):
                    wv8 = wv8_pool.tile([P, DC, 4, 512], F8, tag="wv8",
                                        name="wv8")
                    for dc in range(DC):
                        nc.sync.dma_start(wv8[:, dc, :, :], wv8_d.ap()[dc])
                    wqb_pre = [wb_pool.tile([P, DC, P], BF16, tag="w",
                                            name=f"wqb{m}") for m in range(DC)]
                    wv_tiles = [wv_pool.tile([P, 8, 512], BF16, tag="wv",
                                             name=f"wvb{n}{hb}")
                                for n in range(4) for hb in range(2)]

                    it = 0
                    for g in range(4):
                        for n in range(4):
                            # paced issue: 1 wqb + 1 wvb descriptor per iter
                            if it < DC:
                                nc.scalar.dma_start(wqb_pre[it][:],
                                                    wqb_d.ap()[it])
                            if it < 8:
                                nc.scalar.dma_start(
                                    wv_tiles[it][:],
                                    wvb_d.ap()[it // 2, it % 2])
                            it += 1
                            ps = ps_all.tile([P, 512], F32, tag="ps")
                            for dcp in range(DC // 2):
                                nc.tensor.matmul(
                                    ps[:],
                                    lhsT=xt8[:, 2 * dcp:2 * dcp + 2, ts(g, P)],
                                    rhs=wv8[:, 2 * dcp:2 * dcp + 2, n, :],
                                    start=(dcp == 0), stop=(dcp == DC // 2 - 1),
                                    perf_mode=DR,
                                )
                            sv8 = sv8_pool.tile([P, 512], F8, tag="sv8")
                            nc.vector.tensor_scalar_mul(sv8[:], ps[:],
                                                        PROJ8_STORE)
                            nc.gpsimd.dma_start(vv8_hi[g, :, ts(n, 512)],
                                                sv8[:])
                    gather(vv8_hi, vg8B)

                    for n in range(4):
                        wva, wvb_t = wv_tiles[2 * n], wv_tiles[2 * n + 1]
                        for g in range(4):
                            ps = ps_all.tile([P, 512], F32, tag="ps")
                            for dc in range(DC):
                                w = wva if dc < 8 else wvb_t
                                nc.tensor.matmul(
                                    ps[:], lhsT=xtb[:, dc, ts(g, P)],
                                    rhs=w[:, dc % 8, :],
                                    start=(dc == 0), stop=(dc == DC - 1),
                                )
                            sv = sv_pool.tile([P, 512], BF16, tag="sv")
                            nc.vector.tensor_copy(sv[:], ps[:])
                            sv8 = sv8_pool.tile([P, 512], F8, tag="sv8")
                            nc.vector.tensor_scalar_mul(sv8[:], ps[:], QKS)
                            nc.gpsimd.dma_start(vvb_own[g, :, ts(n, 512)],
                                                sv[:])
                            nc.gpsimd.dma_start(vv8_lo[g, :, ts(n, 512)],
                                                sv8[:])
                    gather(vvb_own, vgb)
                    gather(vv8_lo, vg8A)

                # --- Q c_hi (fp8 DR) straight into SBUF qt8
                wq8_pre = []
                for m in range(DC):
                    wt = w8_pool.tile([P, DC, P], F8, tag="w8", name=f"wq8{m}")
                    nc.sync.dma_start(wt[:], wq8_d.ap()[m])
                    wq8_pre.append(wt)
                for m in range(DC):
                    ps = ps_all.tile([P, 512], F32, tag="ps")
                    for dcp in range(DC // 2):
                        nc.tensor.matmul(
                            ps[:], lhsT=wq8_pre[m][:, 2 * dcp:2 * dcp + 2, :],
                            rhs=xt8[:, 2 * dcp:2 * dcp + 2, :],
                            start=(dcp == 0), stop=(dcp == DC // 2 - 1),
                            perf_mode=DR,
                        )
                    nc.scalar.activation(qt8[:, m, :], ps[:], Copy,
                                         scale=PROJ8_STORE)
                # --- Q c_lo (bf16) via DRAM (reload hides under slot1 scores)
                for m in range(DC):
                    ps = ps_all.tile([P, 512], F32, tag="ps")
                    for dc in range(DC):
                        nc.tensor.matmul(
                            ps[:], lhsT=wqb_pre[m][:, dc, :], rhs=xtb[:, dc, :],
                            start=(dc == 0), stop=(dc == DC - 1),
                        )
                    st = st_pool.tile([P, 512], BF16, tag="st")
                    nc.scalar.copy(st[:], ps[:])
                    nc.gpsimd.dma_start(qTb[:, m, :], st[:])

            # ---------------- phase 2: attention ----------------
            with (
                tc.tile_pool(name="pt0", bufs=1) as pt0_pool,
                tc.tile_pool(name="pt1", bufs=1) as pt1_pool,
                tc.tile_pool(name="mk", bufs=1) as mk_pool,
                tc.tile_pool(name="vb", bufs=1) as vb_pool,
                tc.tile_pool(name="ktb", bufs=6) as ktb_pool,
                tc.tile_pool(name="kt8", bufs=16) as kt8_pool,
                tc.tile_pool(name="qtb", bufs=1) as qtb_pool,
                tc.tile_pool(name="one", bufs=1) as one_pool,
                tc.tile_pool(name="sc", bufs=4) as sc_pool,
                tc.tile_pool(name="ob", bufs=4) as ob_pool,
            ):
                qtb = qtb_pool.tile([P, DC, 512], BF16, tag="qtb", name="qtb")
                for j in range(4):
                    nc.scalar.dma_start(qtb[:, ts(j, 4), :],
                                        qTb[:, ts(j, 4), :])
                mk0 = mk_pool.tile([P, 8, 512], BF16, tag="mk0")
                nc.scalar.dma_start(mk0[:, :4, :], mk0_d.ap()[:, :4, :])
                nc.scalar.dma_start(mk0[:, 4:, :], mk0_d.ap()[:, 4:, :])
                mk1 = mk_pool.tile([P, 8, 512], F8, tag="mk1")
                nc.scalar.dma_start(mk1[:], mk1_d.ap()[:, :, :])
                ones_b = one_pool.tile([P, 1], BF16, tag="onb")
                nc.vector.memset(ones_b[:], 1.0)
                ones8 = one_pool.tile([P, 1], F8, tag="on8")
                nc.vector.memset(ones8[:], QKS)
                bln = one_pool.tile([P, 1], F32, tag="bln")
                nc.vector.memset(bln[:], PS_LN)

                pt1 = pt1_pool.tile([P, DC, 512], F8, tag="pt1")
                pt0 = pt0_pool.tile([P, 8, 512], BF16, tag="pt0")

                # v loads on gpsimd (idle in phase 2); 2 desc per position
                vbig = vb_pool.tile([P, 8, D], BF16, tag="vb", name="vb")
                for j in range(8):
                    src = vgb[j // 4, j % 4]
                    nc.gpsimd.dma_start(vbig[:, j, :1024], src[:, :1024])
                    nc.gpsimd.dma_start(vbig[:, j, 1024:], src[:, 1024:])
                v8 = vb_pool.tile([P, DC, D], F8, tag="v8", name="v8")
                for pos in range(DC):
                    src = (vg8A if pos < 8 else vg8B)[(pos % 8) // 4, pos % 4]
                    nc.gpsimd.dma_start(v8[:, pos, :1024], src[:, :1024])
                    nc.gpsimd.dma_start(v8[:, pos, 1024:], src[:, 1024:])

                # --- slot1 scores (fp8 DR), all 16 key positions
                for pos in range(DC):
                    kt8_t = kt8_pool.tile([P, DC, P], F8, tag="kt8",
                                          name=f"kt8{pos}")
                    ksrc = kg8A if pos < 8 else kg8B
                    o = pos % 8
                    nc.sync.dma_start(kt8_t[:, :8, :],
                                      ksrc[o // 4][:, :8, ts(o % 4, P)])
                    nc.sync.dma_start(kt8_t[:, 8:, :],
                                      ksrc[o // 4][:, 8:, ts(o % 4, P)])
                    ps = ps_all.tile([P, 512], F32, tag="ps")
                    for dcp in range(DC // 2):
                        nc.tensor.matmul(
                            ps[:], lhsT=kt8_t[:, 2 * dcp:2 * dcp + 2, :],
                            rhs=qt8[:, 2 * dcp:2 * dcp + 2, :],
                            start=(dcp == 0), stop=(dcp == DC // 2 - 1),
                            perf_mode=DR,
                        )
                    nc.scalar.activation(pt1[:, pos, :], ps[:], Exp,
                                         scale=EXP8_SCALE, bias=bln[:])

                # --- slot0 scores (bf16), key positions 0-7
                for pos in range(8):
                    ktb_t = ktb_pool.tile([P, DC, P], BF16, tag="ktb",
                                          name=f"ktb{pos}")
                    nc.sync.dma_start(ktb_t[:, :8, :],
                                      kgb[pos // 4][:, :8, ts(pos % 4, P)])
                    nc.sync.dma_start(ktb_t[:, 8:, :],
                                      kgb[pos // 4][:, 8:, ts(pos % 4, P)])
                    ps = ps_all.tile([P, 512], F32, tag="ps")
                    for dc in range(DC):
                        nc.tensor.matmul(
                            ps[:], lhsT=ktb_t[:, dc, :], rhs=qtb[:, dc, :],
                            start=(dc == 0), stop=(dc == DC - 1),
                        )
                    nc.scalar.activation(pt0[:, pos, :], ps[:], Exp,
                                         scale=INV_SQRT_D)

                # --- masks: slot0 all 8; slot1 positions 8-15 only
                for j in range(8):
                    nc.vector.tensor_mul(pt0[:, j, :], pt0[:, j, :],
                                         mk0[:, j, :])
                for j in range(8):
                    nc.vector.tensor_mul(pt1[:, 8 + j, :], pt1[:, 8 + j, :],
                                         mk1[:, j, :])

                # --- slot0 AV (bf16): out rows 0-511
                for qs in range(4):
                    idx = list(range(4)) + list(range(4, 5 + qs))
                    plt = ps_all.tile([P, 512], F32, tag="ps", name="pl0")
                    pl = plt[:, :1]
                    for i, j in enumerate(idx):
                        nc.tensor.matmul(
                            pl[:], lhsT=pt0[:, j, ts(qs, P)], rhs=ones_b[:],
                            start=(i == 0), stop=(i == len(idx) - 1),
                        )
                    rl = sc_pool.tile([P, 1], F32, tag="rl")
                    nc.vector.reciprocal(rl[:], pl[:])
                    for n in range(4):
                        pav = ps_all.tile([P, 512], F32, tag="ps", name="pav0")
                        for i, j in enumerate(idx):
                            nc.tensor.matmul(
                                pav[:], lhsT=pt0[:, j, ts(qs, P)],
                                rhs=vbig[:, j, ts(n, 512)],
                                start=(i == 0), stop=(i == len(idx) - 1),
                            )
                        ob = ob_pool.tile([P, 512], F32, tag="ob")
                        if n % 2 == 0:
                            nc.vector.tensor_scalar_mul(ob[:], pav[:], rl[:])
                        else:
                            nc.scalar.activation(ob[:], pav[:], Copy,
                                                 scale=rl[:])
                        nc.sync.dma_start(
                            out_d.ap()[ds(qs * P, P), ts(n, 512)], ob[:])

                # --- slot1 AV (fp8 DR pairs): out rows 512-1023
                for qs in range(4):
                    idx = (list(range(8)) + list(range(8, 9 + qs))
                           + list(range(12, 16)))
                    runs = _dr_runs(qs)
                    plt = ps_all.tile([P, 512], F32, tag="ps", name="pl1")
                    pl = plt[:, :1]
                    for i, j in enumerate(idx):
                        nc.tensor.matmul(
                            pl[:], lhsT=pt1[:, j, ts(qs, P)], rhs=ones8[:],
                            start=(i == 0), stop=(i == len(idx) - 1),
                        )
                    rl = sc_pool.tile([P, 1], F32, tag="rl")
                    nc.vector.reciprocal(rl[:], pl[:])
                    for n in range(4):
                        pav = ps_all.tile([P, 512], F32, tag="ps", name="pav1")
                        for i, (j, cnt) in enumerate(runs):
                            if cnt == 2:
                                nc.tensor.matmul(
                                    pav[:], lhsT=pt1[:, j:j + 2, ts(qs, P)],
                                    rhs=v8[:, j:j + 2, ts(n, 512)],
                                    start=(i == 0), stop=(i == len(runs) - 1),
                                    perf_mode=DR,
                                )
                            else:
                                nc.tensor.matmul(
                                    pav[:], lhsT=pt1[:, j, ts(qs, P)],
                                    rhs=v8[:, j, ts(n, 512)],
                                    start=(i == 0), stop=(i == len(runs) - 1),
                                )
                        ob = ob_pool.tile([P, 512], F32, tag="ob")
                        if n % 2 == 0:
                            nc.vector.tensor_scalar_mul(ob[:], pav[:], rl[:])
                        else:
                            nc.scalar.activation(ob[:], pav[:], Copy,
                                                 scale=rl[:])
                        nc.sync.dma_start(
                            out_d.ap()[ds(512 + qs * P, P), ts(n, 512)], ob[:])

    nc.compile()
    _CACHED_NC = nc
    return nc


def _host_prep(x, Wq, Wk, Wv):
    """Build per-core input maps (host-side layout + quantization)."""
    E4 = ml_dtypes.float8_e4m3
    BFnp = ml_dtypes.bfloat16

    def wb_layout(W):
        return np.ascontiguousarray(
            W.reshape(DC, P, DC, P).transpose(2, 1, 0, 3)).astype(BFnp)

    def w8_layout(W):
        return np.ascontiguousarray(
            (W * WS).reshape(DC, P, DC, P).transpose(2, 1, 0, 3)).astype(E4)

    wqb_h, wkb_h = wb_layout(Wq), wb_layout(Wk)
    wq8_h, wk8_h = w8_layout(Wq), w8_layout(Wk)
    wvb_h = np.ascontiguousarray(
        Wv.reshape(2, 8, P, 4, 512).transpose(3, 0, 2, 1, 4)).astype(BFnp)
    wv8_h = np.ascontiguousarray(
        (Wv * WS).reshape(DC, P, 4, 512)).astype(E4)

    k_in = np.arange(P, dtype=np.int64)[:, None]        # [P, 1]
    q_in = np.arange(512, dtype=np.int64)[None, :]      # [1, 512]

    def build_masks(h):
        m0 = np.zeros((P, 8, 512), dtype=BFnp)
        for j in range(8):
            m0[:, j, :] = (j * P + k_in) <= (h * 512 + q_in)
        m1 = np.zeros((P, 8, 512), dtype=E4)
        q_off = 1536 if h == 0 else 1024
        for j in range(8):
            tkb = 12 + j if j < 4 else 4 + j
            m1[:, j, :] = (tkb * P + k_in) <= (q_off + q_in)
        return m0, m1

    masks = [build_masks(0), build_masks(1)]

    in_maps = []
    for core in range(NCORES):
        b, h = divmod(core, 2)
        c_lo, c_hi = h, 3 - h
        xt = x[b].T                                      # [D, S] view
        xlo = xt[:, c_lo * 512:(c_lo + 1) * 512]
        xhi = xt[:, c_hi * 512:(c_hi + 1) * 512]
        m0, m1 = masks[h]
        in_maps.append({
            "xtb": np.ascontiguousarray(xlo).astype(BFnp),
            "xt8": np.ascontiguousarray(xhi * XS).astype(E4),
            "wqb": wqb_h, "wkb": wkb_h, "wq8": wq8_h, "wk8": wk8_h,
            "wvb": wvb_h, "wv8": wv8_h, "mk0": m0, "mk1": m1,
        })
    return in_maps


def run(x, Wq, Wk, Wv, trace=False):
    x = np.asarray(x, dtype=np.float32)
    Wq = np.asarray(Wq, dtype=np.float32)
    Wk = np.asarray(Wk, dtype=np.float32)
    Wv = np.asarray(Wv, dtype=np.float32)
    nc = build_nc()
    in_maps = _host_prep(x, Wq, Wk, Wv)
    res = run_bass_kernel_spmd(nc, in_maps, core_ids=list(range(NCORES)),
                               trace=trace)
    out = np.empty((B, S, D), dtype=np.float32)
    for core in range(NCORES):
        b, h = divmod(core, 2)
        c_lo, c_hi = h, 3 - h
        o = res.results[core]["out"]
        out[b, c_lo * 512:(c_lo + 1) * 512] = o[:512]
        out[b, c_hi * 512:(c_hi + 1) * 512] = o[512:]
    return out, res


def kernel(x, Wq, Wk, Wv):
    out, _ = run(x, Wq, Wk, Wv)
    return out


if __name__ == "__main__":
    build_nc()
    print("build + compile OK")
